# revision 28
# baseline (speedup 1.0000x reference)
"""Multi-head causal attention (B=4, T=2048, D=512, H=8) on 8 TRN2 NeuronCores.

Sharding: core c handles batch b = c//2 and head-group hg = c%2 (4 heads,
256 output dims).  No collectives needed — 8 fully independent problems.

Per-core algorithm (matmul inputs bf16, O^T accumulation f32 in PSUM):
  - host passes x^T (D,T) and W^T slices (D, 256) in bf16 + a [128,128]
    triangular causal mask
  - Q^T,K^T projections:  qT[dh2,T] = W2h @ xT, two heads stacked per tile
  - V projection into augmented-V tiles [k-tile 128, 65] (ones column
    appended -> the O^T matmul also produces the softmax denominator row)
  - flash-style: S^T[k,q] = K^T.T @ Q^T per (k-tile, q-block), exp via ACT
    (scale=1/8 folded in; no max subtraction: |scores| < ~4).  Causal:
    diagonal k-tiles only compute q >= k-tile start, triangle-mask multiply
    on boundary blocks.
  - O^T accumulated in PSUM over k-tiles, then PE-transpose + divide by
    denominator -> natural [T,256] -> DMA out

Scheduling (program order == Tile priority): minimal projection prologue
(K chunks + last Q chunk of group 0), then attention units with the
remaining projection/V units woven between score batches as PE filler so
the exp stream (ACT, the critical engine) starts ~13us in and never
starves.  qb runs DESCENDING so attention opens with its PE-densest
stretch — the PE clock (HAM) never throttles down; re-warming from cold
needs a fully-busy 3.4us window that sparse-qb units can't provide.
"""

import numpy as np
import ml_dtypes

T = 2048
D = 512
HG = 4  # heads per core
DH = 64
OUTW = HG * DH  # 256
QB = 512  # q block (columns of S^T tiles)
NQB = T // QB  # 4
NKT = T // 128  # 16 k-tiles
N_CORES = 8

_CACHE = {}


def _build_nc():
    import concourse.bacc as bacc
    import concourse.tile as tile
    import concourse.mybir as mybir
    from concourse.masks import make_identity
    from contextlib import ExitStack

    fp32 = mybir.dt.float32
    bf16 = mybir.dt.bfloat16
    EXP = mybir.ActivationFunctionType.Exp

    nc = bacc.Bacc(None, target_bir_lowering=False)

    xt_d = nc.declare_dram_parameter("xt", [D, T], bf16, isOutput=False)
    wqt_d = nc.declare_dram_parameter("wqt", [D, OUTW], bf16, isOutput=False)
    wkt_d = nc.declare_dram_parameter("wkt", [D, OUTW], bf16, isOutput=False)
    wvt_d = nc.declare_dram_parameter("wvt", [D, OUTW], bf16, isOutput=False)
    cmask_d = nc.declare_dram_parameter("cmask", [128, 128], bf16, isOutput=False)
    out_d = nc.declare_dram_parameter("out", [T, OUTW], fp32, isOutput=True)

    with tile.TileContext(nc) as tc, ExitStack() as ctx:
        const = ctx.enter_context(tc.tile_pool(name="const", bufs=1))
        ps_s = ctx.enter_context(tc.tile_pool(name="ps_s", bufs=2, space="PSUM"))
        pt_pool = ctx.enter_context(tc.tile_pool(name="pt", bufs=6))
        osb_pool = ctx.enter_context(tc.tile_pool(name="osb", bufs=3))
        rec_pool = ctx.enter_context(tc.tile_pool(name="rec", bufs=8))

        # ---- input loads: weights + x split across both HWDGE queues
        def load4(dram, name, width, engs):
            ts = []
            for c in range(4):
                t = const.tile([128, width], bf16, tag=f"{name}{c}", name=f"{name}{c}")
                engs[c % len(engs)].dma_start(
                    out=t[:], in_=dram[c * 128:(c + 1) * 128, :]
                )
                ts.append(t)
            return ts

        wkT = load4(wkt_d, "wkT", OUTW, [nc.sync])
        wqT = load4(wqt_d, "wqT", OUTW, [nc.sync])
        xT = load4(xt_d, "xT", T, [nc.scalar, nc.sync])
        wvT = load4(wvt_d, "wvT", OUTW, [nc.scalar])

        mask_sb = const.tile([128, 128], bf16, name="mask_sb")
        nc.scalar.dma_start(out=mask_sb[:], in_=cmask_d[:])

        ident = const.tile([128, 128], bf16, name="ident")
        make_identity(nc, ident[:])

        # ---- HAM warm-up burst ----
        # The PE clock needs a fully-busy ~3.4us window to reach 2.4GHz.  The
        # real prologue stalls on x^T DMA arrivals (~10us), so it warms late
        # (~22us) and everything before runs at 1.2GHz.  Burn the DMA shadow
        # with dense dummy matmuls instead: ~8 cold MMs trip the ramp, the
        # rest keep the PE busy until the inputs land, so real work starts
        # warm and the PE never idles long enough to re-throttle.
        warm_w = const.tile([128, 128], bf16, name="warm_w")
        warm_x = const.tile([128, QB], bf16, name="warm_x")
        nc.vector.memset(warm_w[:], 0.5)
        nc.vector.memset(warm_x[:], 0.5)
        warm_ps = ps_s.tile([128, QB], fp32, tag="ps", name="warm_ps")
        for _ in range(12):
            nc.tensor.matmul(warm_ps[:], warm_w[:], warm_x[:], start=True, stop=True)

        # ---- persistent SBUF tensors ----
        qT = [const.tile([128, T], bf16, tag=f"qT{g}", name=f"qT{g}") for g in range(2)]
        kT = [const.tile([128, T], bf16, tag=f"kT{g}", name=f"kT{g}") for g in range(2)]
        vaug = const.tile([128, NKT, HG, 65], bf16, name="vaug")
        nc.vector.memset(vaug[:, :, :, 64:65], 1.0)
        out_sb = const.tile([128, NQB * 4, OUTW], fp32, tag="out_sb", name="out_sb")

        def proj_qk(dst, wt, g, qb4):
            ps = ps_s.tile([128, QB], fp32, tag="ps", name="ps")
            for c in range(4):
                nc.tensor.matmul(
                    ps[:],
                    wt[c][:, g * 128:(g + 1) * 128],
                    xT[c][:, qb4 * QB:(qb4 + 1) * QB],
                    start=(c == 0),
                    stop=(c == 3),
                )
            nc.vector.tensor_copy(dst[g][:, qb4 * QB:(qb4 + 1) * QB], ps[:])

        def proj_v(tt):
            ps = ps_s.tile([128, OUTW], fp32, tag="ps", name="ps")
            for c in range(4):
                nc.tensor.matmul(
                    ps[:],
                    xT[c][:, tt * 128:(tt + 1) * 128],
                    wvT[c][:, 0:OUTW],
                    start=(c == 0),
                    stop=(c == 3),
                )
            nc.vector.tensor_copy(
                vaug[:, tt, :, 0:64],
                ps[:].rearrange("p (h d) -> p h d", h=HG),
            )

        def attn_unit(qb, h, filler=None):
            """One (head, q-block) attention unit.  filler(i) is called
            between score-batch i and its exp to weave in other PE work."""
            g, po = h // 2, 64 * (h % 2)
            ot = ps_s.tile([128, QB], fp32, tag="ot", name="ot")
            last_kt = qb * 4 + 3

            def score_mm(st_ap, kt, q0, width):
                nc.tensor.matmul(
                    st_ap,
                    kT[g][po:po + 64, kt * 128:(kt + 1) * 128],
                    qT[g][po:po + 64, qb * QB + q0: qb * QB + q0 + width],
                    start=True,
                    stop=True,
                )

            def ot_mm(kt, pt_ap, q0, width):
                nc.tensor.matmul(
                    ot[0:65, q0:q0 + width],
                    vaug[:, kt, h, :],
                    pt_ap,
                    start=(kt == 0),
                    stop=(kt == last_kt),
                )

            # batch list: off-diagonal pairs, then two diagonal pairs with
            # causal restriction (widths 512/384 and 256/128)
            batches = [((kt0, 0, QB), (kt0 + 1, 0, QB))
                       for kt0 in range(0, qb * 4, 2)]
            batches.append(((qb * 4 + 0, 0, QB), (qb * 4 + 1, 128, QB - 128)))
            batches.append(((qb * 4 + 2, 256, QB - 256), (qb * 4 + 3, 384, QB - 384)))

            for i, ((kta, qa, wa), (ktb, qbk, wb)) in enumerate(batches):
                diag = i >= len(batches) - 2
                st = ps_s.tile([128, 2 * QB], fp32, tag="st", name="st")
                score_mm(st[:, 0:wa], kta, qa, wa)
                score_mm(st[:, wa:wa + wb], ktb, qbk, wb)
                if filler:
                    filler(i)
                pt = pt_pool.tile([128, 2 * QB], bf16, tag="pt", name="pt")
                nc.scalar.activation(
                    pt[:, 0:wa + wb], st[:, 0:wa + wb], func=EXP, scale=0.125
                )
                if diag:
                    nc.vector.tensor_mul(pt[:, 0:128], pt[:, 0:128], mask_sb[:])
                    nc.vector.tensor_mul(
                        pt[:, wa:wa + 128], pt[:, wa:wa + 128], mask_sb[:]
                    )
                ot_mm(kta, pt[:, 0:wa], qa, wa)
                ot_mm(ktb, pt[:, wa:wa + wb], qbk, wb)

            # normalize + transpose to natural layout (bf16: transpose
            # LDWEIGHTS/matmul run at full rate, fp32 would be 2-4x slower)
            osb = osb_pool.tile([65, QB], bf16, tag="osb", name="osb")
            nc.vector.tensor_copy(osb[:], ot[0:65, :])
            for j4 in range(4):
                tp = ps_s.tile([128, 128], bf16, tag="ps", name="tp")
                nc.tensor.transpose(
                    tp[:, 0:65],
                    osb[:, j4 * 128:(j4 + 1) * 128],
                    ident[0:65, 0:65],
                )
                rec = rec_pool.tile([128, 1], fp32, tag="rec", name="rec")
                nc.vector.reciprocal(rec[:], tp[:, 64:65])
                nc.vector.tensor_scalar_mul(
                    out_sb[:, qb * 4 + j4, h * 64:(h + 1) * 64],
                    tp[:, 0:64],
                    rec[:],
                )

        def stream_out(qb):
            for j4 in range(4):
                tt = qb * 4 + j4
                nc.sync.dma_start(
                    out=out_d[tt * 128:(tt + 1) * 128, :], in_=out_sb[:, tt, :]
                )

        # ---- schedule ----
        # minimal prologue: only what (3, h0)'s first score batches need;
        # remaining kT g0 chunks arrive from fillers >= 2 batches early
        proj_qk(kT, wkT, 0, 0)
        proj_qk(qT, wqT, 0, 3)

        # Two passes: all group-0 heads (qb desc), then all group-1 heads.
        # Projection/V units spread thinly across the g0 pass's ACT slack:
        # V + remaining kT-g0 chunks must live in (3,h0) (just-in-time for
        # its own batches); qT-g0 chunks in (3,h1); group-1 K/Q across the
        # (2,h0)/(2,h1) units.
        fill_30 = {i: [("v", 2 * i), ("v", 2 * i + 1)] for i in range(8)}
        fill_30[0].append(("k", 0, 1))
        fill_30[1].append(("k", 0, 2))
        fill_30[2].append(("k", 0, 3))
        fill_31 = {0: [("q", 0, 2)], 1: [("q", 0, 1)], 2: [("q", 0, 0)]}
        fill_20 = {0: [("k", 1, 0)], 1: [("k", 1, 1)],
                   2: [("k", 1, 2)], 3: [("k", 1, 3)]}
        fill_21 = {0: [("q", 1, 3)], 1: [("q", 1, 2)],
                   2: [("q", 1, 1)], 3: [("q", 1, 0)]}

        def run_fill(plan, i):
            for item in plan.get(i, ()):
                if item[0] == "v":
                    proj_v(item[1])
                elif item[0] == "k":
                    proj_qk(kT, wkT, item[1], item[2])
                else:
                    proj_qk(qT, wqT, item[1], item[2])

        # bridge the burst-to-attention PE hole (MID re-throttle at ~18us)
        for _ in range(6):
            nc.tensor.matmul(warm_ps[:], warm_w[:], warm_x[:], start=True, stop=True)

        attn_unit(3, 0, filler=lambda i: run_fill(fill_30, i))
        attn_unit(3, 1, filler=lambda i: run_fill(fill_31, i))
        attn_unit(2, 0, filler=lambda i: run_fill(fill_20, i))
        attn_unit(2, 1, filler=lambda i: run_fill(fill_21, i))
        for qb in (1, 0):
            for h in (0, 1):
                attn_unit(qb, h)
        for qb in (3, 2, 1, 0):
            for h in (2, 3):
                attn_unit(qb, h)
            stream_out(qb)

    nc.finalize()
    return nc


def _get_nc():
    if "nc" not in _CACHE:
        _CACHE["nc"] = _build_nc()
    return _CACHE["nc"]


def _make_cmask():
    # triangle: mask[p, f] = 1.0 iff p <= f
    p = np.arange(128)[:, None]
    f = np.arange(128)[None, :]
    return (p <= f).astype(ml_dtypes.bfloat16)


def _make_in_maps(x, Wq, Wk, Wv):
    bf = ml_dtypes.bfloat16
    cmask = _make_cmask()
    in_maps = []
    for c in range(N_CORES):
        b, hg = c // 2, c % 2
        r0 = hg * OUTW
        in_maps.append({
            "xt": np.ascontiguousarray(x[b].T).astype(bf),
            "wqt": np.ascontiguousarray(Wq[r0:r0 + OUTW].T).astype(bf),
            "wkt": np.ascontiguousarray(Wk[r0:r0 + OUTW].T).astype(bf),
            "wvt": np.ascontiguousarray(Wv[r0:r0 + OUTW].T).astype(bf),
            "cmask": cmask,
        })
    return in_maps


def kernel(x, Wq, Wk, Wv):
    from concourse.bass_utils import run_bass_kernel_spmd

    nc = _get_nc()
    in_maps = _make_in_maps(x, Wq, Wk, Wv)
    res = run_bass_kernel_spmd(nc, in_maps, core_ids=list(range(N_CORES)))

    B = x.shape[0]
    out = np.empty((B, T, D), dtype=np.float32)
    for c in range(N_CORES):
        b, hg = c // 2, c % 2
        out[b, :, hg * OUTW:(hg + 1) * OUTW] = res.results[c]["out"]
    return out


# revision 29
# speedup vs baseline: 1.0001x; 1.0001x over previous
"""Multi-head causal attention (B=4, T=2048, D=512, H=8) on 8 TRN2 NeuronCores.

Sharding: core c handles batch b = c//2 and head-group hg = c%2 (4 heads,
256 output dims).  No collectives needed — 8 fully independent problems.

Per-core algorithm (matmul inputs bf16, O^T accumulation f32 in PSUM):
  - host passes x^T (D,T) and W^T slices (D, 256) in bf16 + a [128,128]
    triangular causal mask
  - Q^T,K^T projections:  qT[dh2,T] = W2h @ xT, two heads stacked per tile
  - V projection into augmented-V tiles [k-tile 128, 65] (ones column
    appended -> the O^T matmul also produces the softmax denominator row)
  - flash-style: S^T[k,q] = K^T.T @ Q^T per (k-tile, q-block), exp via ACT
    (scale=1/8 folded in; no max subtraction: |scores| < ~4).  Causal:
    diagonal k-tiles only compute q >= k-tile start, triangle-mask multiply
    on boundary blocks.
  - O^T accumulated in PSUM over k-tiles, then PE-transpose + divide by
    denominator -> natural [T,256] -> DMA out

Scheduling (program order == Tile priority): minimal projection prologue
(K chunks + last Q chunk of group 0), then attention units with the
remaining projection/V units woven between score batches as PE filler so
the exp stream (ACT, the critical engine) starts ~13us in and never
starves.  qb runs DESCENDING so attention opens with its PE-densest
stretch — the PE clock (HAM) never throttles down; re-warming from cold
needs a fully-busy 3.4us window that sparse-qb units can't provide.
"""

import numpy as np
import ml_dtypes

T = 2048
D = 512
HG = 4  # heads per core
DH = 64
OUTW = HG * DH  # 256
QB = 512  # q block (columns of S^T tiles)
NQB = T // QB  # 4
NKT = T // 128  # 16 k-tiles
N_CORES = 8

_CACHE = {}


def _build_nc():
    import concourse.bacc as bacc
    import concourse.tile as tile
    import concourse.mybir as mybir
    from concourse.masks import make_identity
    from contextlib import ExitStack

    fp32 = mybir.dt.float32
    bf16 = mybir.dt.bfloat16
    EXP = mybir.ActivationFunctionType.Exp

    nc = bacc.Bacc(None, target_bir_lowering=False)

    xt_d = nc.declare_dram_parameter("xt", [D, T], bf16, isOutput=False)
    wqt_d = nc.declare_dram_parameter("wqt", [D, OUTW], bf16, isOutput=False)
    wkt_d = nc.declare_dram_parameter("wkt", [D, OUTW], bf16, isOutput=False)
    wvt_d = nc.declare_dram_parameter("wvt", [D, OUTW], bf16, isOutput=False)
    cmask_d = nc.declare_dram_parameter("cmask", [128, 128], bf16, isOutput=False)
    out_d = nc.declare_dram_parameter("out", [T, OUTW], fp32, isOutput=True)

    with tile.TileContext(nc) as tc, ExitStack() as ctx:
        const = ctx.enter_context(tc.tile_pool(name="const", bufs=1))
        ps_s = ctx.enter_context(tc.tile_pool(name="ps_s", bufs=2, space="PSUM"))
        pt_pool = ctx.enter_context(tc.tile_pool(name="pt", bufs=6))
        osb_pool = ctx.enter_context(tc.tile_pool(name="osb", bufs=3))
        rec_pool = ctx.enter_context(tc.tile_pool(name="rec", bufs=8))

        # ---- input loads: weights + x split across both HWDGE queues
        def load4(dram, name, width, engs):
            ts = []
            for c in range(4):
                t = const.tile([128, width], bf16, tag=f"{name}{c}", name=f"{name}{c}")
                engs[c % len(engs)].dma_start(
                    out=t[:], in_=dram[c * 128:(c + 1) * 128, :]
                )
                ts.append(t)
            return ts

        wkT = load4(wkt_d, "wkT", OUTW, [nc.sync])
        wqT = load4(wqt_d, "wqT", OUTW, [nc.sync])
        xT = load4(xt_d, "xT", T, [nc.scalar, nc.sync])
        wvT = load4(wvt_d, "wvT", OUTW, [nc.scalar])

        mask_sb = const.tile([128, 128], bf16, name="mask_sb")
        nc.scalar.dma_start(out=mask_sb[:], in_=cmask_d[:])

        ident = const.tile([128, 128], bf16, name="ident")
        make_identity(nc, ident[:])

        # ---- HAM warm-up burst ----
        # The PE clock needs a fully-busy ~3.4us window to reach 2.4GHz.  The
        # real prologue stalls on x^T DMA arrivals (~10us), so it warms late
        # (~22us) and everything before runs at 1.2GHz.  Burn the DMA shadow
        # with dense dummy matmuls instead: ~8 cold MMs trip the ramp, the
        # rest keep the PE busy until the inputs land, so real work starts
        # warm and the PE never idles long enough to re-throttle.
        warm_w = const.tile([128, 128], bf16, name="warm_w")
        warm_x = const.tile([128, QB], bf16, name="warm_x")
        nc.vector.memset(warm_w[:], 0.5)
        nc.vector.memset(warm_x[:], 0.5)
        warm_ps = ps_s.tile([128, QB], fp32, tag="ps", name="warm_ps")
        for _ in range(12):
            nc.tensor.matmul(warm_ps[:], warm_w[:], warm_x[:], start=True, stop=True)

        # ---- persistent SBUF tensors ----
        qT = [const.tile([128, T], bf16, tag=f"qT{g}", name=f"qT{g}") for g in range(2)]
        kT = [const.tile([128, T], bf16, tag=f"kT{g}", name=f"kT{g}") for g in range(2)]
        vaug = const.tile([128, NKT, HG, 65], bf16, name="vaug")
        nc.vector.memset(vaug[:, :, :, 64:65], 1.0)
        out_sb = const.tile([128, NQB * 4, OUTW], fp32, tag="out_sb", name="out_sb")

        def proj_qk(dst, wt, g, qb4):
            ps = ps_s.tile([128, QB], fp32, tag="ps", name="ps")
            for c in range(4):
                nc.tensor.matmul(
                    ps[:],
                    wt[c][:, g * 128:(g + 1) * 128],
                    xT[c][:, qb4 * QB:(qb4 + 1) * QB],
                    start=(c == 0),
                    stop=(c == 3),
                )
            nc.vector.tensor_copy(dst[g][:, qb4 * QB:(qb4 + 1) * QB], ps[:])

        def proj_v(tt):
            ps = ps_s.tile([128, OUTW], fp32, tag="ps", name="ps")
            for c in range(4):
                nc.tensor.matmul(
                    ps[:],
                    xT[c][:, tt * 128:(tt + 1) * 128],
                    wvT[c][:, 0:OUTW],
                    start=(c == 0),
                    stop=(c == 3),
                )
            nc.vector.tensor_copy(
                vaug[:, tt, :, 0:64],
                ps[:].rearrange("p (h d) -> p h d", h=HG),
            )

        def attn_unit(qb, h, filler=None):
            """One (head, q-block) attention unit.  filler(i) is called
            between score-batch i and its exp to weave in other PE work."""
            g, po = h // 2, 64 * (h % 2)
            ot = ps_s.tile([128, QB], fp32, tag="ot", name="ot")
            last_kt = qb * 4 + 3

            def score_mm(st_ap, kt, q0, width):
                nc.tensor.matmul(
                    st_ap,
                    kT[g][po:po + 64, kt * 128:(kt + 1) * 128],
                    qT[g][po:po + 64, qb * QB + q0: qb * QB + q0 + width],
                    start=True,
                    stop=True,
                )

            def ot_mm(kt, pt_ap, q0, width):
                nc.tensor.matmul(
                    ot[0:65, q0:q0 + width],
                    vaug[:, kt, h, :],
                    pt_ap,
                    start=(kt == 0),
                    stop=(kt == last_kt),
                )

            # batch list: off-diagonal pairs, then two diagonal pairs with
            # causal restriction (widths 512/384 and 256/128)
            batches = [((kt0, 0, QB), (kt0 + 1, 0, QB))
                       for kt0 in range(0, qb * 4, 2)]
            batches.append(((qb * 4 + 0, 0, QB), (qb * 4 + 1, 128, QB - 128)))
            batches.append(((qb * 4 + 2, 256, QB - 256), (qb * 4 + 3, 384, QB - 384)))

            for i, ((kta, qa, wa), (ktb, qbk, wb)) in enumerate(batches):
                diag = i >= len(batches) - 2
                st = ps_s.tile([128, 2 * QB], fp32, tag="st", name="st")
                score_mm(st[:, 0:wa], kta, qa, wa)
                score_mm(st[:, wa:wa + wb], ktb, qbk, wb)
                if filler:
                    filler(i)
                pt = pt_pool.tile([128, 2 * QB], bf16, tag="pt", name="pt")
                nc.scalar.activation(
                    pt[:, 0:wa + wb], st[:, 0:wa + wb], func=EXP, scale=0.125
                )
                if diag:
                    nc.vector.tensor_mul(pt[:, 0:128], pt[:, 0:128], mask_sb[:])
                    nc.vector.tensor_mul(
                        pt[:, wa:wa + 128], pt[:, wa:wa + 128], mask_sb[:]
                    )
                ot_mm(kta, pt[:, 0:wa], qa, wa)
                ot_mm(ktb, pt[:, wa:wa + wb], qbk, wb)

            # normalize + transpose to natural layout (bf16: transpose
            # LDWEIGHTS/matmul run at full rate, fp32 would be 2-4x slower)
            osb = osb_pool.tile([65, QB], bf16, tag="osb", name="osb")
            nc.vector.tensor_copy(osb[:], ot[0:65, :])
            for j4 in range(4):
                tp = ps_s.tile([128, 128], bf16, tag="ps", name="tp")
                nc.tensor.transpose(
                    tp[:, 0:65],
                    osb[:, j4 * 128:(j4 + 1) * 128],
                    ident[0:65, 0:65],
                )
                rec = rec_pool.tile([128, 1], fp32, tag="rec", name="rec")
                nc.vector.reciprocal(rec[:], tp[:, 64:65])
                nc.vector.tensor_scalar_mul(
                    out_sb[:, qb * 4 + j4, h * 64:(h + 1) * 64],
                    tp[:, 0:64],
                    rec[:],
                )

        def stream_out(qb):
            for j4 in range(4):
                tt = qb * 4 + j4
                nc.sync.dma_start(
                    out=out_d[tt * 128:(tt + 1) * 128, :], in_=out_sb[:, tt, :]
                )

        # ---- schedule ----
        # minimal prologue: only what (3, h0)'s first score batches need;
        # remaining kT g0 chunks arrive from fillers >= 2 batches early
        proj_qk(kT, wkT, 0, 0)
        proj_qk(qT, wqT, 0, 3)

        # Two passes: all group-0 heads (qb desc), then all group-1 heads.
        # Projection/V units spread thinly across the g0 pass's ACT slack:
        # V + remaining kT-g0 chunks must live in (3,h0) (just-in-time for
        # its own batches); qT-g0 chunks in (3,h1); group-1 K/Q across the
        # (2,h0)/(2,h1) units.
        fill_30 = {i: [("v", 2 * i), ("v", 2 * i + 1)] for i in range(8)}
        fill_30[0].append(("k", 0, 1))
        fill_30[1].append(("k", 0, 2))
        fill_30[2].append(("k", 0, 3))
        fill_31 = {0: [("q", 0, 2)], 1: [("q", 0, 1)], 2: [("q", 0, 0)]}
        fill_20 = {0: [("k", 1, 0)], 1: [("k", 1, 1)],
                   2: [("k", 1, 2)], 3: [("k", 1, 3)]}
        fill_21 = {0: [("q", 1, 3)], 1: [("q", 1, 2)],
                   2: [("q", 1, 1)], 3: [("q", 1, 0)]}

        def run_fill(plan, i):
            for item in plan.get(i, ()):
                if item[0] == "v":
                    proj_v(item[1])
                elif item[0] == "k":
                    proj_qk(kT, wkT, item[1], item[2])
                else:
                    proj_qk(qT, wqT, item[1], item[2])

        attn_unit(3, 0, filler=lambda i: run_fill(fill_30, i))
        attn_unit(3, 1, filler=lambda i: run_fill(fill_31, i))
        attn_unit(2, 0, filler=lambda i: run_fill(fill_20, i))
        attn_unit(2, 1, filler=lambda i: run_fill(fill_21, i))
        for qb in (1, 0):
            for h in (0, 1):
                attn_unit(qb, h)
        for qb in (3, 2, 1, 0):
            for h in (2, 3):
                attn_unit(qb, h)
            stream_out(qb)

    nc.finalize()
    return nc


def _get_nc():
    if "nc" not in _CACHE:
        _CACHE["nc"] = _build_nc()
    return _CACHE["nc"]


def _make_cmask():
    # triangle: mask[p, f] = 1.0 iff p <= f
    p = np.arange(128)[:, None]
    f = np.arange(128)[None, :]
    return (p <= f).astype(ml_dtypes.bfloat16)


def _make_in_maps(x, Wq, Wk, Wv):
    bf = ml_dtypes.bfloat16
    cmask = _make_cmask()
    in_maps = []
    for c in range(N_CORES):
        b, hg = c // 2, c % 2
        r0 = hg * OUTW
        in_maps.append({
            "xt": np.ascontiguousarray(x[b].T).astype(bf),
            "wqt": np.ascontiguousarray(Wq[r0:r0 + OUTW].T).astype(bf),
            "wkt": np.ascontiguousarray(Wk[r0:r0 + OUTW].T).astype(bf),
            "wvt": np.ascontiguousarray(Wv[r0:r0 + OUTW].T).astype(bf),
            "cmask": cmask,
        })
    return in_maps


def kernel(x, Wq, Wk, Wv):
    from concourse.bass_utils import run_bass_kernel_spmd

    nc = _get_nc()
    in_maps = _make_in_maps(x, Wq, Wk, Wv)
    res = run_bass_kernel_spmd(nc, in_maps, core_ids=list(range(N_CORES)))

    B = x.shape[0]
    out = np.empty((B, T, D), dtype=np.float32)
    for c in range(N_CORES):
        b, hg = c // 2, c % 2
        out[b, :, hg * OUTW:(hg + 1) * OUTW] = res.results[c]["out"]
    return out
